# revision 53
# baseline (speedup 1.0000x reference)
"""Causal self-attention (QK-RMSNorm + rotary, H=16, D=1024, B=2, T=2048) on 8 NeuronCores.

Sharding: core c handles batch b = c // 4 and heads 4*(c%4) .. 4*(c%4)+3,
processed as two head PAIRS. Each core computes the qkv projection for its
heads, causal attention, and a row-parallel slice of the output projection;
the host sums the 4 partial outputs per batch element.

v2 design (vs the fp32r baseline):
- All matmul operands in bf16 (x, weights, q, k, v, p, y): halves DMA/SBUF.
- Rotary via a cross-partition DMA shift (rot(q) = shift(q) * signed-sin)
  instead of a second full projection: saves ~57k PE cycles.
- RMS scale applied to raw q/k before rope (rope is norm-preserving and
  commutes with per-head scalars); stats batched into one ln+exp per pair.
- Attention: full-512-wide score matmuls, exp batched in [128,1024] pairs
  (amortizes ACT's per-instruction bubble), causal masking via a Pool-side
  tri multiply on the diagonal squares, y-matmuls restricted to [o:512].
- Softmax sums ride the v ones-column (partition 64/63 of yacc) as in the
  baseline; division is per i-block so the out-projection overlaps the tail
  of attention.
- Engine balance: PE does matmuls only; ACT does exps+stats; DVE does
  PSUM->SBUF copies and PSUM-operand muls; Pool (gpsimd) does SBUF-only
  muls/adds (rope combine, tri); DMA does the rotary shift and v transpose.
"""
import sys
sys.path.insert(0, '/opt/trn_rl_repo')

import numpy as np
import ml_dtypes
from contextlib import ExitStack

import concourse.bass as bass
import concourse.tile as tile
from concourse import bacc, mybir
from concourse.bass_utils import run_bass_kernel_spmd

F32 = mybir.dt.float32
BF = mybir.dt.bfloat16
AF = mybir.ActivationFunctionType

N_HEAD = 16
D_MODEL = 1024
D_HEAD = 64
B, T = 2, 2048
N_CORES = 8
HL = 4              # heads per core
KT = D_MODEL // 128  # 8 contraction tiles
NCH = T // 512      # 4 token chunks
NIB = T // 512      # 4 i-blocks
NTT = T // 128      # 16 j-tiles
SCALE = D_HEAD ** -0.5

_cached = {}


def _build(debug_dump=False):
    nc = bacc.Bacc("TRN2", target_bir_lowering=False, debug=False,
                   num_devices=N_CORES)

    # ---- DRAM I/O ----------------------------------------------------------
    xT = nc.dram_tensor("xT", [D_MODEL, T], BF, kind="ExternalInput").ap()
    wA = nc.dram_tensor("wA", [2, D_MODEL, 384], BF, kind="ExternalInput").ap()
    cosT = nc.dram_tensor("cosT", [128, T], BF, kind="ExternalInput").ap()
    sinNegT = nc.dram_tensor("sinNegT", [128, T], BF,
                             kind="ExternalInput").ap()
    trimask = nc.dram_tensor("trimask", [128, 128], BF,
                             kind="ExternalInput").ap()
    selqk = nc.dram_tensor("selqk", [128, 8, 16], BF,
                           kind="ExternalInput").ap()
    selbc = nc.dram_tensor("selbc", [16, 8, 128], BF,
                           kind="ExternalInput").ap()
    sel4 = nc.dram_tensor("sel4", [4, 2, 128], BF, kind="ExternalInput").ap()
    zpadQ = nc.dram_tensor("zpadQ", [64, T], BF, kind="ExternalInput").ap()
    onescol = nc.dram_tensor("onescol", [128, HL * NTT], BF,
                             kind="ExternalInput").ap()
    wpP = nc.dram_tensor("wpP", [2, 128, 1024], BF, kind="ExternalInput").ap()
    out = nc.dram_tensor("out", [T, D_MODEL], F32, kind="ExternalOutput").ap()
    if debug_dump:
        dbg = {
            "d_qsb": nc.dram_tensor("d_qsb", [2, 128, T], BF,
                                    kind="ExternalOutput").ap(),
            "d_qTz": nc.dram_tensor("d_qTz", [HL, 128, T], BF,
                                    kind="ExternalOutput").ap(),
            "d_kT": nc.dram_tensor("d_kT", [2, 128, T], BF,
                                   kind="ExternalOutput").ap(),
            "d_v": nc.dram_tensor("d_v", [128, 5200], BF,
                                  kind="ExternalOutput").ap(),
            "d_sums": nc.dram_tensor("d_sums", [4, NIB, 512], F32,
                                     kind="ExternalOutput").ap(),
            "d_yP": nc.dram_tensor("d_yP", [2, 128, T], BF,
                                   kind="ExternalOutput").ap(),
        }

    with tile.TileContext(nc) as tc, ExitStack() as ctx:
        ctx.enter_context(nc.allow_low_precision(
            reason="bf16 matmuls/intermediates; tolerance is 2e-2"))

        cpool = ctx.enter_context(tc.tile_pool(name="consts", bufs=1))
        work = ctx.enter_context(tc.tile_pool(name="work", bufs=2))
        ps_s = ctx.enter_context(tc.tile_pool(name="pss", bufs=2,
                                              space="PSUM"))

        # ---- persistent SBUF -----------------------------------------------
        x_sb = cpool.tile([128, KT, T], BF)
        wA_sb = [cpool.tile([128, KT, 384], BF, name=f"wA{p}") for p in (0, 1)]
        cos_sb = cpool.tile([128, T], BF)
        sinNeg_sb = cpool.tile([128, T], BF)
        tri_sb = cpool.tile([128, 128], BF)
        selqk_sb = cpool.tile([128, 8, 16], BF)
        selbc_sb = cpool.tile([16, 8, 128], BF)
        sel4_sb = cpool.tile([4, 2, 128], BF)
        wpP_sb = [cpool.tile([128, 1024], BF, name=f"wpP{p}") for p in (0, 1)]
        qTz = [cpool.tile([128, T], BF, name=f"qTz{h}") for h in range(HL)]
        kT_sb = [cpool.tile([128, T], BF, name=f"kT{p}") for p in (0, 1)]
        # padded past HL*NTT*65+64 so the [p, 2, 1040] transpose-dst view of
        # the last head pair stays in bounds
        v_sb = cpool.tile([128, 5200], BF)
        v3 = v_sb[:, 0:HL * NTT * 65].rearrange("p (g o) -> p g o", o=65)
        yP = [cpool.tile([128, T], BF, name=f"yP{p}") for p in (0, 1)]
        qsbF = [[cpool.tile([128, T], BF, name=f"qsb{p}{m}") for m in (0, 1)]
                for p in (0, 1)]
        qshF = [[cpool.tile([128, T], BF, name=f"qsh{p}{m}") for m in (0, 1)]
                for p in (0, 1)]
        rinv_sb = [cpool.tile([16, 512], BF, name=f"rinv{p}") for p in (0, 1)]
        sums_sb = cpool.tile([4, NIB, 512], F32)
        rinvy_sb = cpool.tile([4, NIB, 512], BF)

        # ---- preamble DMAs -------------------------------------------------
        xr = xT.rearrange("(k p) t -> p k t", p=128)
        nc.sync.dma_start(wA_sb[0][:],
                          wA[0].rearrange("(k p) c -> p k c", p=128))
        nc.sync.dma_start(x_sb[:, :, 0:512], xr[:, :, 0:512])
        for ch in range(1, NCH):
            nc.scalar.dma_start(x_sb[:, :, ch * 512:(ch + 1) * 512],
                                xr[:, :, ch * 512:(ch + 1) * 512])
        nc.sync.dma_start(selqk_sb[:], selqk[:])
        nc.sync.dma_start(wA_sb[1][:],
                          wA[1].rearrange("(k p) c -> p k c", p=128))
        nc.sync.dma_start(cos_sb[:], cosT[:])
        nc.sync.dma_start(sinNeg_sb[:], sinNegT[:])
        nc.sync.dma_start(tri_sb[:], trimask[:])
        nc.sync.dma_start(selbc_sb[:], selbc[:])
        nc.sync.dma_start(sel4_sb[:], sel4[:])
        for h in range(HL):
            half = slice(64, 128) if h % 2 == 0 else slice(0, 64)
            nc.sync.dma_start(qTz[h][half, :], zpadQ[:])
        nc.sync.dma_start(v3[:, :, 64:65], onescol.unsqueeze(2))
        for p in (0, 1):
            nc.sync.dma_start(wpP_sb[p][:], wpP[p])

        ySG_store = {}

        # ---- unit emitters -------------------------------------------------
        def proj_unit(ps_w, hp, m, ch, st_tile):
            cs = slice(ch * 512, (ch + 1) * 512)
            acc = ps_w.tile([128, 512], F32, tag="pa", bufs=2, name="acc")
            for k in range(KT):
                nc.tensor.matmul(acc[:],
                                 wA_sb[hp][:, k, m * 128:(m + 1) * 128],
                                 x_sb[:, k, cs], start=(k == 0),
                                 stop=(k == KT - 1))
            if m < 2:
                dst = qsbF[hp][m][:, cs]
                nc.scalar.copy(dst, acc[:])
                if ch == NCH - 1:
                    # all 4 chunks landed: batched square + 4 stats matmuls
                    sqF = qshF[hp][m]
                    nc.vector.tensor_mul(sqF[:], qsbF[hp][m][:],
                                         qsbF[hp][m][:])
                    for c2 in range(NCH):
                        idx = m * 4 + c2
                        nc.tensor.matmul(
                            st_tile[:], selqk_sb[:, idx, :],
                            sqF[:, c2 * 512:(c2 + 1) * 512],
                            start=(idx == 0), stop=(idx == 7))
            else:
                vdst = work.tile([128, 512], BF, tag="vsb", bufs=3,
                                 name="vdst")
                nc.vector.tensor_copy(vdst[:], acc[:])
                for s4 in range(4):
                    jt = ch * 4 + s4
                    gA = (2 * hp) * NTT + jt
                    vstg = work.tile([128, 128], BF, tag="vstg", bufs=3,
                                     name="vstg")
                    nc.sync.dma_start_transpose(
                        vstg[:], vdst[:, s4 * 128:(s4 + 1) * 128])
                    vv = v_sb[:, gA * 65:gA * 65 + 2080] \
                        .rearrange("p (h x) -> p h x", h=2)[:, :, 0:64]
                    nc.gpsimd.tensor_copy(
                        vv, vstg[:].rearrange("p (h x) -> p h x", h=2))

        def lnexp(hp, st_tile):
            lnt = work.tile([16, 512], F32, tag="lnt", bufs=1, name="lnt")
            nc.scalar.activation(lnt[:], st_tile[:], AF.Ln, scale=1.0 / 64.0)
            nc.scalar.activation(rinv_sb[hp][:], lnt[:], AF.Exp, scale=-0.5)

        def fin_chunks(hp, bcpool, bctag, bcbufs):
            """Chunk closures: rms-scale in place, rotary shift + combine.

            Full-T granularity: t1 overwrites qsbF, t2 overwrites qshF
            (both dead afterwards), so no scratch tiles are needed."""
            chunks = []
            for m in (0, 1):
                for ch in range(NCH):
                    def bc_scale(m=m, ch=ch):
                        cs = slice(ch * 512, (ch + 1) * 512)
                        bc = bcpool.tile([128, 512], F32, tag=bctag,
                                         bufs=bcbufs, name="bc")
                        nc.tensor.matmul(bc[:], selbc_sb[:, m * 4 + ch, :],
                                         rinv_sb[hp][:], start=True,
                                         stop=True)
                        nc.vector.tensor_mul(qsbF[hp][m][:, cs],
                                             qsbF[hp][m][:, cs], bc[:])
                    chunks.append(bc_scale)

                def shifts(m=m):
                    for blk in range(4):
                        d0 = blk * 32
                        s0 = (blk ^ 1) * 32
                        nc.sync.dma_start(qshF[hp][m][d0:d0 + 32, :],
                                          qsbF[hp][m][s0:s0 + 32, :])
                chunks.append(shifts)

            def rope_mul(m, which):
                if which == 0:
                    nc.vector.tensor_mul(qsbF[hp][m][:], qsbF[hp][m][:],
                                         cos_sb[:])
                else:
                    nc.vector.tensor_mul(qshF[hp][m][:], qshF[hp][m][:],
                                         sinNeg_sb[:])

            def rope_add(m):
                if m == 0:
                    nc.vector.tensor_add(qTz[2 * hp][0:64, :],
                                         qsbF[hp][0][0:64, :],
                                         qshF[hp][0][0:64, :])
                    nc.vector.tensor_add(qTz[2 * hp + 1][64:128, :],
                                         qsbF[hp][0][64:128, :],
                                         qshF[hp][0][64:128, :])
                else:
                    nc.vector.tensor_add(kT_sb[hp][:], qsbF[hp][1][:],
                                         qshF[hp][1][:])

            chunks.append(lambda: rope_mul(0, 1))
            chunks.append(lambda: rope_mul(1, 1))
            chunks.append(lambda: rope_mul(0, 0))
            chunks.append(lambda: rope_add(0))
            chunks.append(lambda: rope_mul(1, 0))
            chunks.append(lambda: rope_add(1))
            return chunks

        def attn_gen(ps_o, hp, hh, ib):
            """Generator form: yields after each score/exp batch so two heads
            can interleave on the PE (hides the exp latency)."""
            h_l = 2 * hp + hh
            njt = 4 * (ib + 1)
            ibs = ib * 512
            yacc = ps_o.tile([128, 512], F32, tag="y", bufs=2, name="yacc")

            def ymms(pr, pt):
                for half in (0, 1):
                    jt = 2 * pr + half
                    o = max(0, jt * 128 - ibs)
                    g = h_l * NTT + jt
                    if hh == 0:
                        vau = v_sb[:, g * 65:g * 65 + 128]
                    else:
                        vau = v_sb[:, g * 65 - 64:g * 65 + 64]
                    nc.tensor.matmul(yacc[:, o:512], vau,
                                     pt[:, half * 512 + o:half * 512 + 512],
                                     start=(jt == 0), stop=(jt == njt - 1))

            prev = None
            for pr in range(njt // 2):
                sp = ps_s.tile([128, 1024], F32, tag="s", name="sp")
                for half in (0, 1):
                    jt = 2 * pr + half
                    o_s = jt * 128 - ibs
                    o_s = o_s if 0 < o_s <= 256 else 0
                    nc.tensor.matmul(
                        sp[:, half * 512 + o_s:(half + 1) * 512],
                        kT_sb[hp][:, jt * 128:(jt + 1) * 128],
                        qTz[h_l][:, ibs + o_s:ibs + 512],
                        start=True, stop=True)
                pt = work.tile([128, 1024], BF, tag="p", bufs=4, name="pt")
                nc.scalar.activation(pt[:], sp[:], AF.Exp, scale=SCALE)
                for half in (0, 1):
                    jt = 2 * pr + half
                    o = jt * 128 - ibs
                    if o >= 0:
                        lo = half * 512 + o
                        nc.vector.tensor_mul(pt[:, lo:lo + 128],
                                             pt[:, lo:lo + 128], tri_sb[:])
                if prev is not None:
                    ymms(*prev)
                prev = (pr, pt)
                yield
            ymms(*prev)

            ySG = work.tile([128, 512], F32, tag="ysg", bufs=12, name="ySG")
            nc.vector.tensor_copy(ySG[:], yacc[:])
            srow = 64 if hh == 0 else 63
            nc.sync.dma_start(sums_sb[2 * hp + hh:2 * hp + hh + 1, ib, :],
                              ySG[srow:srow + 1, :])
            ySG_store[(hp, hh, ib)] = ySG

        def attn_pair(ps_o, hp, ib):
            g0 = attn_gen(ps_o, hp, 0, ib)
            g1 = attn_gen(ps_o, hp, 1, ib)
            for _ in range(4 * (ib + 1) // 2 + 1):
                next(g0, None)
                next(g1, None)

        def recip_unit(ib):
            # one reciprocal per i-block covers both pairs; split into 4
            # chunks so attention's DVE ops can slot between them
            for q in range(4):
                nc.vector.reciprocal(rinvy_sb[:, ib, q * 128:(q + 1) * 128],
                                     sums_sb[:, ib, q * 128:(q + 1) * 128])

        def ydiv_unit(ps_o, hp, ib):
            ibs = ib * 512
            bc2 = ps_o.tile([128, 512], F32, tag="oa", bufs=2, name="bc2")
            nc.tensor.matmul(bc2[:], sel4_sb[:, hp, :], rinvy_sb[:, ib, :],
                             start=True, stop=True)
            e = ySG_store[(hp, 0, ib)]
            o_ = ySG_store[(hp, 1, ib)]
            nc.vector.tensor_mul(yP[hp][0:64, ibs:ibs + 512], e[0:64, :],
                                 bc2[0:64, :])
            nc.vector.tensor_mul(yP[hp][64:128, ibs:ibs + 512],
                                 o_[64:128, :], bc2[64:128, :])

        def o_unit(ps_o, ib):
            for mt in range(4 * ib, 4 * ib + 4):
                ost = work.tile([128, 1024], F32, tag="ost", name="ost")
                for oc in (0, 1):
                    oa = ps_o.tile([128, 512], F32, tag="oa", bufs=2,
                                   name="oa")
                    for t in (0, 1):
                        nc.tensor.matmul(oa[:],
                                         yP[t][:, mt * 128:(mt + 1) * 128],
                                         wpP_sb[t][:, oc * 512:(oc + 1) * 512],
                                         start=(t == 0), stop=(t == 1))
                    dst = ost[:, oc * 512:(oc + 1) * 512]
                    if ib == 0 and oc == 1:
                        nc.scalar.copy(dst, oa[:])
                    else:
                        nc.vector.tensor_copy(dst, oa[:])
                nc.sync.dma_start(out[mt * 128:(mt + 1) * 128, :], ost[:])

        # ---- emission ------------------------------------------------------
        with tc.tile_pool(name="psw", bufs=1, space="PSUM") as ps_w:
            st0 = ps_w.tile([16, 512], F32, tag="st", name="st0")
            for m in range(3):
                for ch in range(NCH):
                    proj_unit(ps_w, 0, m, ch, st0)
            lnexp(0, st0)

            # fin(p0) interleaved with P(p1)
            st1 = ps_w.tile([16, 512], F32, tag="st", name="st1")
            p1_units = [(m, ch) for m in range(3) for ch in range(NCH)]
            f0 = fin_chunks(0, ps_w, "bcw", 1)
            fi = 0
            for i, (m, ch) in enumerate(p1_units):
                proj_unit(ps_w, 1, m, ch, st1)
                take = ((i + 1) * len(f0)) // len(p1_units) - fi
                for _ in range(take):
                    f0[fi]()
                    fi += 1
            lnexp(1, st1)

        # attention; psw's banks are free, so yacc is double-buffered and
        # the oa/bc2 ring fits alongside the score ring
        with tc.tile_pool(name="pso", bufs=2, space="PSUM") as ps_o:
            # A(pair0) interleaved with fin(p1)
            f1 = fin_chunks(1, ps_o, "oa", 2)
            fi = 0
            for i, ib in enumerate((3, 2, 1, 0)):
                take = ((i + 1) * len(f1)) // NIB - fi
                for _ in range(take):
                    f1[fi]()
                    fi += 1
                attn_pair(ps_o, 0, ib)

            # pair1 attention + division + out-projection, lagged one i-block
            for i, ib in enumerate((3, 2, 1, 0)):
                attn_pair(ps_o, 1, ib)
                recip_unit(ib)
                if i >= 1:
                    prev = 3 - (i - 1)
                    ydiv_unit(ps_o, 0, prev)
                    ydiv_unit(ps_o, 1, prev)
                    o_unit(ps_o, prev)
            ydiv_unit(ps_o, 0, 0)
            ydiv_unit(ps_o, 1, 0)
            o_unit(ps_o, 0)

        if debug_dump:
            for p in (0, 1):
                nc.sync.dma_start(dbg["d_qsb"][p], qsbF[p][0][:])
                nc.sync.dma_start(dbg["d_kT"][p], kT_sb[p][:])
                nc.sync.dma_start(dbg["d_yP"][p], yP[p][:])
            nc.sync.dma_start(dbg["d_sums"], sums_sb[:])
            for h in range(HL):
                nc.sync.dma_start(dbg["d_qTz"][h], qTz[h][:])
            nc.sync.dma_start(dbg["d_v"], v_sb[:])

    nc.compile()
    return nc


def _host_inputs(x, w_attn, w_proj):
    """Build the 8 per-core input maps (bf16 device tensors)."""
    bf = ml_dtypes.bfloat16
    inv_freq = 1.0 / (10000.0 ** (np.arange(0, D_HEAD, 2, dtype=np.float32)
                                  / D_HEAD))
    t = np.arange(T, dtype=np.float32)
    freqs = np.einsum('i,j->ij', t, inv_freq)            # [T, 32]
    emb = np.concatenate([freqs, freqs], 1)              # [T, 64]
    cos64 = np.cos(emb).T                                # [64, T]
    sin64 = np.sin(emb).T
    sgn = np.where(np.arange(64) < 32, -1.0, 1.0)[:, None].astype(np.float32)
    sinNeg64 = sin64 * sgn
    cosT = np.concatenate([cos64, cos64], 0).astype(bf)  # [128, T]
    sinNegT = np.concatenate([sinNeg64, sinNeg64], 0).astype(bf)

    tri = (np.arange(128)[:, None] <= np.arange(128)[None, :]).astype(bf)

    selqk = np.zeros((128, 8, 16), np.float32)
    selbc = np.zeros((16, 8, 128), np.float32)
    for m in range(2):
        for ch in range(NCH):
            u = m * 4 + ch
            for p in range(128):
                c = 4 * ch + 2 * m + (1 if p >= 64 else 0)
                selqk[p, u, c] = 1.0
                selbc[c, u, p] = 1.0
    sel4 = np.zeros((4, 2, 128), np.float32)
    for p in (0, 1):
        sel4[2 * p, p, 0:64] = 1.0
        sel4[2 * p + 1, p, 64:128] = 1.0

    zpadQ = np.zeros((64, T), np.float32)
    onescol = np.ones((128, HL * NTT), np.float32)

    wq = w_attn[:D_MODEL]
    wk = w_attn[D_MODEL:2 * D_MODEL]
    wv_full = w_attn[2 * D_MODEL:]

    in_maps = []
    for c in range(N_CORES):
        b, hg = c // 4, c % 4
        wA = np.zeros((2, D_MODEL, 384), np.float32)
        for hp in (0, 1):
            hs = slice((hg * 4 + 2 * hp) * D_HEAD,
                       (hg * 4 + 2 * hp + 2) * D_HEAD)     # 128 rows
            wA[hp, :, 0:128] = wq[hs].T
            wA[hp, :, 128:256] = wk[hs].T
            wA[hp, :, 256:384] = wv_full[hs].T
        wp_c = [w_proj[:, (hg * 4 + j) * D_HEAD:(hg * 4 + j + 1) * D_HEAD].T
                for j in range(HL)]
        wpP = np.stack([np.concatenate([wp_c[0], wp_c[1]], 0),
                        np.concatenate([wp_c[2], wp_c[3]], 0)])

        in_maps.append({
            "xT": np.ascontiguousarray(x[b].T).astype(bf),
            "wA": wA.astype(bf),
            "cosT": cosT, "sinNegT": sinNegT, "trimask": tri,
            "selqk": selqk.astype(bf), "selbc": selbc.astype(bf),
            "sel4": sel4.astype(bf),
            "zpadQ": zpadQ.astype(bf), "onescol": onescol.astype(bf),
            "wpP": wpP.astype(bf),
        })
    return in_maps


def kernel(x, w_attn, w_proj, _want_results=False):
    x = np.asarray(x, dtype=np.float32)
    w_attn = np.asarray(w_attn, dtype=np.float32)
    w_proj = np.asarray(w_proj, dtype=np.float32)

    if "nc" not in _cached:
        _cached["nc"] = _build()
    nc = _cached["nc"]

    in_maps = _host_inputs(x, w_attn, w_proj)
    res = run_bass_kernel_spmd(nc, in_maps, list(range(N_CORES)))

    full = np.zeros((B, T, D_MODEL), np.float32)
    for c in range(N_CORES):
        full[c // 4] += res.results[c]["out"]
    if _want_results:
        return full, res
    return full


# revision 54
# speedup vs baseline: 1.1392x; 1.1392x over previous
"""Causal self-attention (QK-RMSNorm + rotary, H=16, D=1024, B=2, T=2048) on 8 NeuronCores.

Sharding: core c handles batch b = c // 4 and heads 4*(c%4) .. 4*(c%4)+3,
processed as two head PAIRS. Each core computes the qkv projection for its
heads, causal attention, and a row-parallel slice of the output projection;
the host sums the 4 partial outputs per batch element.

v2 design (vs the fp32r baseline):
- All matmul operands in bf16 (x, weights, q, k, v, p, y): halves DMA/SBUF.
- Rotary via a cross-partition DMA shift (rot(q) = shift(q) * signed-sin)
  instead of a second full projection: saves ~57k PE cycles.
- RMS scale applied to raw q/k before rope (rope is norm-preserving and
  commutes with per-head scalars); stats batched into one ln+exp per pair.
- Attention: full-512-wide score matmuls, exp batched in [128,1024] pairs
  (amortizes ACT's per-instruction bubble), causal masking via a Pool-side
  tri multiply on the diagonal squares, y-matmuls restricted to [o:512].
- Softmax sums ride the v ones-column (partition 64/63 of yacc) as in the
  baseline; division is per i-block so the out-projection overlaps the tail
  of attention.
- Engine balance: PE does matmuls only; ACT does exps+stats; DVE does
  PSUM->SBUF copies and PSUM-operand muls; Pool (gpsimd) does SBUF-only
  muls/adds (rope combine, tri); DMA does the rotary shift and v transpose.
"""
import sys
sys.path.insert(0, '/opt/trn_rl_repo')

import numpy as np
import ml_dtypes
from contextlib import ExitStack

import concourse.bass as bass
import concourse.tile as tile
from concourse import bacc, mybir
from concourse.bass_utils import run_bass_kernel_spmd

F32 = mybir.dt.float32
BF = mybir.dt.bfloat16
AF = mybir.ActivationFunctionType

N_HEAD = 16
D_MODEL = 1024
D_HEAD = 64
B, T = 2, 2048
N_CORES = 8
HL = 4              # heads per core
KT = D_MODEL // 128  # 8 contraction tiles
NCH = T // 512      # 4 token chunks
NIB = T // 512      # 4 i-blocks
NTT = T // 128      # 16 j-tiles
SCALE = D_HEAD ** -0.5

_cached = {}


def _build(debug_dump=False):
    nc = bacc.Bacc("TRN2", target_bir_lowering=False, debug=False,
                   num_devices=N_CORES)

    # ---- DRAM I/O ----------------------------------------------------------
    xT = nc.dram_tensor("xT", [D_MODEL, T], BF, kind="ExternalInput").ap()
    wA = nc.dram_tensor("wA", [2, D_MODEL, 384], BF, kind="ExternalInput").ap()
    cosT = nc.dram_tensor("cosT", [128, T], BF, kind="ExternalInput").ap()
    sinNegT = nc.dram_tensor("sinNegT", [128, T], BF,
                             kind="ExternalInput").ap()
    trimask = nc.dram_tensor("trimask", [128, 128], BF,
                             kind="ExternalInput").ap()
    selqk = nc.dram_tensor("selqk", [128, 8, 16], BF,
                           kind="ExternalInput").ap()
    selbc = nc.dram_tensor("selbc", [16, 8, 128], BF,
                           kind="ExternalInput").ap()
    sel4 = nc.dram_tensor("sel4", [4, 2, 128], BF, kind="ExternalInput").ap()
    zpadQ = nc.dram_tensor("zpadQ", [64, T], BF, kind="ExternalInput").ap()
    onescol = nc.dram_tensor("onescol", [128, HL * NTT], BF,
                             kind="ExternalInput").ap()
    wpP = nc.dram_tensor("wpP", [2, 128, 1024], BF, kind="ExternalInput").ap()
    out = nc.dram_tensor("out", [T, D_MODEL], F32, kind="ExternalOutput").ap()
    if debug_dump:
        dbg = {
            "d_qsb": nc.dram_tensor("d_qsb", [2, 128, T], BF,
                                    kind="ExternalOutput").ap(),
            "d_qTz": nc.dram_tensor("d_qTz", [HL, 128, T], BF,
                                    kind="ExternalOutput").ap(),
            "d_kT": nc.dram_tensor("d_kT", [2, 128, T], BF,
                                   kind="ExternalOutput").ap(),
            "d_v": nc.dram_tensor("d_v", [128, 5200], BF,
                                  kind="ExternalOutput").ap(),
            "d_sums": nc.dram_tensor("d_sums", [4, NIB, 512], F32,
                                     kind="ExternalOutput").ap(),
            "d_yP": nc.dram_tensor("d_yP", [2, 128, T], BF,
                                   kind="ExternalOutput").ap(),
        }

    with tile.TileContext(nc) as tc, ExitStack() as ctx:
        ctx.enter_context(nc.allow_low_precision(
            reason="bf16 matmuls/intermediates; tolerance is 2e-2"))

        cpool = ctx.enter_context(tc.tile_pool(name="consts", bufs=1))
        work = ctx.enter_context(tc.tile_pool(name="work", bufs=2))
        ps_s = ctx.enter_context(tc.tile_pool(name="pss", bufs=2,
                                              space="PSUM"))

        # ---- persistent SBUF -----------------------------------------------
        x_sb = cpool.tile([128, KT, T], BF)
        wA_sb = [cpool.tile([128, KT, 384], BF, name=f"wA{p}") for p in (0, 1)]
        cos_sb = cpool.tile([128, T], BF)
        sinNeg_sb = cpool.tile([128, T], BF)
        tri_sb = cpool.tile([128, 128], BF)
        selqk_sb = cpool.tile([128, 8, 16], BF)
        selbc_sb = cpool.tile([16, 8, 128], BF)
        sel4_sb = cpool.tile([4, 2, 128], BF)
        wpP_sb = [cpool.tile([128, 1024], BF, name=f"wpP{p}") for p in (0, 1)]
        qTz = [cpool.tile([128, T], BF, name=f"qTz{h}") for h in range(HL)]
        kT_sb = [cpool.tile([128, T], BF, name=f"kT{p}") for p in (0, 1)]
        # padded past HL*NTT*65+64 so the [p, 2, 1040] transpose-dst view of
        # the last head pair stays in bounds
        v_sb = cpool.tile([128, 5200], BF)
        v3 = v_sb[:, 0:HL * NTT * 65].rearrange("p (g o) -> p g o", o=65)
        yP = [cpool.tile([128, T], BF, name=f"yP{p}") for p in (0, 1)]
        qsbF = [[cpool.tile([128, T], BF, name=f"qsb{p}{m}") for m in (0, 1)]
                for p in (0, 1)]
        qshF = [[cpool.tile([128, T], BF, name=f"qsh{p}{m}") for m in (0, 1)]
                for p in (0, 1)]
        rinv_sb = [cpool.tile([16, 512], BF, name=f"rinv{p}") for p in (0, 1)]
        sums_sb = cpool.tile([4, NIB, 512], F32)
        rinvy_sb = cpool.tile([4, NIB, 512], BF)

        # ---- preamble DMAs -------------------------------------------------
        xr = xT.rearrange("(k p) t -> p k t", p=128)
        nc.sync.dma_start(wA_sb[0][:],
                          wA[0].rearrange("(k p) c -> p k c", p=128))
        for ch in range(NCH):
            nc.sync.dma_start(x_sb[:, :, ch * 512:(ch + 1) * 512],
                              xr[:, :, ch * 512:(ch + 1) * 512])
        nc.sync.dma_start(selqk_sb[:], selqk[:])
        nc.sync.dma_start(wA_sb[1][:],
                          wA[1].rearrange("(k p) c -> p k c", p=128))
        nc.sync.dma_start(cos_sb[:], cosT[:])
        nc.sync.dma_start(sinNeg_sb[:], sinNegT[:])
        nc.sync.dma_start(tri_sb[:], trimask[:])
        nc.sync.dma_start(selbc_sb[:], selbc[:])
        nc.sync.dma_start(sel4_sb[:], sel4[:])
        for h in range(HL):
            half = slice(64, 128) if h % 2 == 0 else slice(0, 64)
            nc.sync.dma_start(qTz[h][half, :], zpadQ[:])
        nc.sync.dma_start(v3[:, :, 64:65], onescol.unsqueeze(2))
        for p in (0, 1):
            nc.sync.dma_start(wpP_sb[p][:], wpP[p])

        ySG_store = {}

        # ---- unit emitters -------------------------------------------------
        def proj_unit(ps_w, hp, m, ch, st_tile):
            cs = slice(ch * 512, (ch + 1) * 512)
            acc = ps_w.tile([128, 512], F32, tag="pa", bufs=2, name="acc")
            for k in range(KT):
                nc.tensor.matmul(acc[:],
                                 wA_sb[hp][:, k, m * 128:(m + 1) * 128],
                                 x_sb[:, k, cs], start=(k == 0),
                                 stop=(k == KT - 1))
            if m < 2:
                dst = qsbF[hp][m][:, cs]
                nc.scalar.copy(dst, acc[:])
                if ch == NCH - 1:
                    # all 4 chunks landed: batched square + 4 stats matmuls
                    sqF = qshF[hp][m]
                    nc.vector.tensor_mul(sqF[:], qsbF[hp][m][:],
                                         qsbF[hp][m][:])
                    for c2 in range(NCH):
                        idx = m * 4 + c2
                        nc.tensor.matmul(
                            st_tile[:], selqk_sb[:, idx, :],
                            sqF[:, c2 * 512:(c2 + 1) * 512],
                            start=(idx == 0), stop=(idx == 7))
            else:
                vdst = work.tile([128, 512], BF, tag="vsb", bufs=3,
                                 name="vdst")
                nc.vector.tensor_copy(vdst[:], acc[:])
                for s4 in range(4):
                    jt = ch * 4 + s4
                    gA = (2 * hp) * NTT + jt
                    vstg = work.tile([128, 128], BF, tag="vstg", bufs=3,
                                     name="vstg")
                    nc.sync.dma_start_transpose(
                        vstg[:], vdst[:, s4 * 128:(s4 + 1) * 128])
                    vv = v_sb[:, gA * 65:gA * 65 + 2080] \
                        .rearrange("p (h x) -> p h x", h=2)[:, :, 0:64]
                    nc.gpsimd.tensor_copy(
                        vv, vstg[:].rearrange("p (h x) -> p h x", h=2))

        def lnexp(hp, st_tile):
            lnt = work.tile([16, 512], F32, tag="lnt", bufs=1, name="lnt")
            nc.scalar.activation(lnt[:], st_tile[:], AF.Ln, scale=1.0 / 64.0)
            nc.scalar.activation(rinv_sb[hp][:], lnt[:], AF.Exp, scale=-0.5)

        def fin_chunks(hp, bcpool, bctag, bcbufs):
            """Chunk closures: rms-scale in place, rotary shift + combine.

            Full-T granularity: t1 overwrites qsbF, t2 overwrites qshF
            (both dead afterwards), so no scratch tiles are needed."""
            chunks = []
            for m in (0, 1):
                for ch in range(NCH):
                    def bc_scale(m=m, ch=ch):
                        cs = slice(ch * 512, (ch + 1) * 512)
                        bc = bcpool.tile([128, 512], F32, tag=bctag,
                                         bufs=bcbufs, name="bc")
                        nc.tensor.matmul(bc[:], selbc_sb[:, m * 4 + ch, :],
                                         rinv_sb[hp][:], start=True,
                                         stop=True)
                        nc.vector.tensor_mul(qsbF[hp][m][:, cs],
                                             qsbF[hp][m][:, cs], bc[:])
                    chunks.append(bc_scale)

                def shifts(m=m):
                    for blk in range(4):
                        d0 = blk * 32
                        s0 = (blk ^ 1) * 32
                        nc.sync.dma_start(qshF[hp][m][d0:d0 + 32, :],
                                          qsbF[hp][m][s0:s0 + 32, :])
                chunks.append(shifts)

            def rope_mul(m, which):
                if which == 0:
                    nc.vector.tensor_mul(qsbF[hp][m][:], qsbF[hp][m][:],
                                         cos_sb[:])
                else:
                    nc.vector.tensor_mul(qshF[hp][m][:], qshF[hp][m][:],
                                         sinNeg_sb[:])

            def rope_add(m):
                if m == 0:
                    nc.vector.tensor_add(qTz[2 * hp][0:64, :],
                                         qsbF[hp][0][0:64, :],
                                         qshF[hp][0][0:64, :])
                    nc.vector.tensor_add(qTz[2 * hp + 1][64:128, :],
                                         qsbF[hp][0][64:128, :],
                                         qshF[hp][0][64:128, :])
                else:
                    nc.vector.tensor_add(kT_sb[hp][:], qsbF[hp][1][:],
                                         qshF[hp][1][:])

            chunks.append(lambda: rope_mul(0, 1))
            chunks.append(lambda: rope_mul(1, 1))
            chunks.append(lambda: rope_mul(0, 0))
            chunks.append(lambda: rope_add(0))
            chunks.append(lambda: rope_mul(1, 0))
            chunks.append(lambda: rope_add(1))
            return chunks

        def attn_gen(ps_o, hp, hh, ib):
            """Generator form: yields after each score/exp batch so two heads
            can interleave on the PE (hides the exp latency)."""
            h_l = 2 * hp + hh
            njt = 4 * (ib + 1)
            ibs = ib * 512
            yacc = ps_o.tile([128, 512], F32, tag="y", bufs=2, name="yacc")

            def ymms(pr, pt):
                for half in (0, 1):
                    jt = 2 * pr + half
                    o = max(0, jt * 128 - ibs)
                    g = h_l * NTT + jt
                    if hh == 0:
                        vau = v_sb[:, g * 65:g * 65 + 128]
                    else:
                        vau = v_sb[:, g * 65 - 64:g * 65 + 64]
                    nc.tensor.matmul(yacc[:, o:512], vau,
                                     pt[:, half * 512 + o:half * 512 + 512],
                                     start=(jt == 0), stop=(jt == njt - 1))

            prev = None
            for pr in range(njt // 2):
                sp = ps_s.tile([128, 1024], F32, tag="s", name="sp")
                for half in (0, 1):
                    jt = 2 * pr + half
                    o_s = jt * 128 - ibs
                    o_s = o_s if 0 < o_s <= 256 else 0
                    nc.tensor.matmul(
                        sp[:, half * 512 + o_s:(half + 1) * 512],
                        kT_sb[hp][:, jt * 128:(jt + 1) * 128],
                        qTz[h_l][:, ibs + o_s:ibs + 512],
                        start=True, stop=True)
                pt = work.tile([128, 1024], BF, tag="p", bufs=4, name="pt")
                nc.scalar.activation(pt[:], sp[:], AF.Exp, scale=SCALE)
                for half in (0, 1):
                    jt = 2 * pr + half
                    o = jt * 128 - ibs
                    if o >= 0:
                        lo = half * 512 + o
                        nc.vector.tensor_mul(pt[:, lo:lo + 128],
                                             pt[:, lo:lo + 128], tri_sb[:])
                if prev is not None:
                    ymms(*prev)
                prev = (pr, pt)
                yield
            ymms(*prev)

            ySG = work.tile([128, 512], F32, tag="ysg", bufs=12, name="ySG")
            nc.vector.tensor_copy(ySG[:], yacc[:])
            srow = 64 if hh == 0 else 63
            nc.sync.dma_start(sums_sb[2 * hp + hh:2 * hp + hh + 1, ib, :],
                              ySG[srow:srow + 1, :])
            ySG_store[(hp, hh, ib)] = ySG

        def attn_pair(ps_o, hp, ib):
            g0 = attn_gen(ps_o, hp, 0, ib)
            g1 = attn_gen(ps_o, hp, 1, ib)
            for _ in range(4 * (ib + 1) // 2 + 1):
                next(g0, None)
                next(g1, None)

        def recip_unit(ib):
            # one reciprocal per i-block covers both pairs; split into 4
            # chunks so attention's DVE ops can slot between them
            for q in range(4):
                nc.vector.reciprocal(rinvy_sb[:, ib, q * 128:(q + 1) * 128],
                                     sums_sb[:, ib, q * 128:(q + 1) * 128])

        def ydiv_unit(ps_o, hp, ib):
            ibs = ib * 512
            bc2 = ps_o.tile([128, 512], F32, tag="oa", bufs=2, name="bc2")
            nc.tensor.matmul(bc2[:], sel4_sb[:, hp, :], rinvy_sb[:, ib, :],
                             start=True, stop=True)
            e = ySG_store[(hp, 0, ib)]
            o_ = ySG_store[(hp, 1, ib)]
            nc.vector.tensor_mul(yP[hp][0:64, ibs:ibs + 512], e[0:64, :],
                                 bc2[0:64, :])
            nc.vector.tensor_mul(yP[hp][64:128, ibs:ibs + 512],
                                 o_[64:128, :], bc2[64:128, :])

        def o_unit(ps_o, ib):
            for mt in range(4 * ib, 4 * ib + 4):
                ost = work.tile([128, 1024], F32, tag="ost", name="ost")
                for oc in (0, 1):
                    oa = ps_o.tile([128, 512], F32, tag="oa", bufs=2,
                                   name="oa")
                    for t in (0, 1):
                        nc.tensor.matmul(oa[:],
                                         yP[t][:, mt * 128:(mt + 1) * 128],
                                         wpP_sb[t][:, oc * 512:(oc + 1) * 512],
                                         start=(t == 0), stop=(t == 1))
                    dst = ost[:, oc * 512:(oc + 1) * 512]
                    if ib == 0 and oc == 1:
                        nc.scalar.copy(dst, oa[:])
                    else:
                        nc.vector.tensor_copy(dst, oa[:])
                nc.sync.dma_start(out[mt * 128:(mt + 1) * 128, :], ost[:])

        # ---- emission ------------------------------------------------------
        with tc.tile_pool(name="psw", bufs=1, space="PSUM") as ps_w:
            st0 = ps_w.tile([16, 512], F32, tag="st", name="st0")
            for m in range(3):
                for ch in range(NCH):
                    proj_unit(ps_w, 0, m, ch, st0)
            lnexp(0, st0)

            # fin(p0) interleaved with P(p1)
            st1 = ps_w.tile([16, 512], F32, tag="st", name="st1")
            p1_units = [(m, ch) for m in range(3) for ch in range(NCH)]
            f0 = fin_chunks(0, ps_w, "bcw", 1)
            fi = 0
            for i, (m, ch) in enumerate(p1_units):
                proj_unit(ps_w, 1, m, ch, st1)
                take = ((i + 1) * len(f0)) // len(p1_units) - fi
                for _ in range(take):
                    f0[fi]()
                    fi += 1
            lnexp(1, st1)

        # attention; psw's banks are free, so yacc is double-buffered and
        # the oa/bc2 ring fits alongside the score ring
        with tc.tile_pool(name="pso", bufs=2, space="PSUM") as ps_o:
            # A(pair0) interleaved with fin(p1)
            f1 = fin_chunks(1, ps_o, "oa", 2)
            fi = 0
            for i, ib in enumerate((3, 2, 1, 0)):
                take = ((i + 1) * len(f1)) // NIB - fi
                for _ in range(take):
                    f1[fi]()
                    fi += 1
                attn_pair(ps_o, 0, ib)

            # pair1 attention + division + out-projection, lagged one i-block
            for i, ib in enumerate((3, 2, 1, 0)):
                attn_pair(ps_o, 1, ib)
                recip_unit(ib)
                if i >= 1:
                    prev = 3 - (i - 1)
                    ydiv_unit(ps_o, 0, prev)
                    ydiv_unit(ps_o, 1, prev)
                    o_unit(ps_o, prev)
            ydiv_unit(ps_o, 0, 0)
            ydiv_unit(ps_o, 1, 0)
            o_unit(ps_o, 0)

        if debug_dump:
            for p in (0, 1):
                nc.sync.dma_start(dbg["d_qsb"][p], qsbF[p][0][:])
                nc.sync.dma_start(dbg["d_kT"][p], kT_sb[p][:])
                nc.sync.dma_start(dbg["d_yP"][p], yP[p][:])
            nc.sync.dma_start(dbg["d_sums"], sums_sb[:])
            for h in range(HL):
                nc.sync.dma_start(dbg["d_qTz"][h], qTz[h][:])
            nc.sync.dma_start(dbg["d_v"], v_sb[:])

    nc.compile()
    return nc


def _host_inputs(x, w_attn, w_proj):
    """Build the 8 per-core input maps (bf16 device tensors)."""
    bf = ml_dtypes.bfloat16
    inv_freq = 1.0 / (10000.0 ** (np.arange(0, D_HEAD, 2, dtype=np.float32)
                                  / D_HEAD))
    t = np.arange(T, dtype=np.float32)
    freqs = np.einsum('i,j->ij', t, inv_freq)            # [T, 32]
    emb = np.concatenate([freqs, freqs], 1)              # [T, 64]
    cos64 = np.cos(emb).T                                # [64, T]
    sin64 = np.sin(emb).T
    sgn = np.where(np.arange(64) < 32, -1.0, 1.0)[:, None].astype(np.float32)
    sinNeg64 = sin64 * sgn
    cosT = np.concatenate([cos64, cos64], 0).astype(bf)  # [128, T]
    sinNegT = np.concatenate([sinNeg64, sinNeg64], 0).astype(bf)

    tri = (np.arange(128)[:, None] <= np.arange(128)[None, :]).astype(bf)

    selqk = np.zeros((128, 8, 16), np.float32)
    selbc = np.zeros((16, 8, 128), np.float32)
    for m in range(2):
        for ch in range(NCH):
            u = m * 4 + ch
            for p in range(128):
                c = 4 * ch + 2 * m + (1 if p >= 64 else 0)
                selqk[p, u, c] = 1.0
                selbc[c, u, p] = 1.0
    sel4 = np.zeros((4, 2, 128), np.float32)
    for p in (0, 1):
        sel4[2 * p, p, 0:64] = 1.0
        sel4[2 * p + 1, p, 64:128] = 1.0

    zpadQ = np.zeros((64, T), np.float32)
    onescol = np.ones((128, HL * NTT), np.float32)

    wq = w_attn[:D_MODEL]
    wk = w_attn[D_MODEL:2 * D_MODEL]
    wv_full = w_attn[2 * D_MODEL:]

    in_maps = []
    for c in range(N_CORES):
        b, hg = c // 4, c % 4
        wA = np.zeros((2, D_MODEL, 384), np.float32)
        for hp in (0, 1):
            hs = slice((hg * 4 + 2 * hp) * D_HEAD,
                       (hg * 4 + 2 * hp + 2) * D_HEAD)     # 128 rows
            wA[hp, :, 0:128] = wq[hs].T
            wA[hp, :, 128:256] = wk[hs].T
            wA[hp, :, 256:384] = wv_full[hs].T
        wp_c = [w_proj[:, (hg * 4 + j) * D_HEAD:(hg * 4 + j + 1) * D_HEAD].T
                for j in range(HL)]
        wpP = np.stack([np.concatenate([wp_c[0], wp_c[1]], 0),
                        np.concatenate([wp_c[2], wp_c[3]], 0)])

        in_maps.append({
            "xT": np.ascontiguousarray(x[b].T).astype(bf),
            "wA": wA.astype(bf),
            "cosT": cosT, "sinNegT": sinNegT, "trimask": tri,
            "selqk": selqk.astype(bf), "selbc": selbc.astype(bf),
            "sel4": sel4.astype(bf),
            "zpadQ": zpadQ.astype(bf), "onescol": onescol.astype(bf),
            "wpP": wpP.astype(bf),
        })
    return in_maps


def kernel(x, w_attn, w_proj, _want_results=False):
    x = np.asarray(x, dtype=np.float32)
    w_attn = np.asarray(w_attn, dtype=np.float32)
    w_proj = np.asarray(w_proj, dtype=np.float32)

    if "nc" not in _cached:
        _cached["nc"] = _build()
    nc = _cached["nc"]

    in_maps = _host_inputs(x, w_attn, w_proj)
    res = run_bass_kernel_spmd(nc, in_maps, list(range(N_CORES)))

    full = np.zeros((B, T, D_MODEL), np.float32)
    for c in range(N_CORES):
        full[c // 4] += res.results[c]["out"]
    if _want_results:
        return full, res
    return full


# revision 55
# speedup vs baseline: 1.2261x; 1.0762x over previous
"""Causal self-attention (QK-RMSNorm + rotary, H=16, D=1024, B=2, T=2048) on 8 NeuronCores.

Sharding: core c handles batch b = c // 4 and heads 4*(c%4) .. 4*(c%4)+3,
processed as two head PAIRS. Each core computes the qkv projection for its
heads, causal attention, and a row-parallel slice of the output projection;
the host sums the 4 partial outputs per batch element.

v2 design (vs the fp32r baseline):
- All matmul operands in bf16 (x, weights, q, k, v, p, y): halves DMA/SBUF.
- Rotary via a cross-partition DMA shift (rot(q) = shift(q) * signed-sin)
  instead of a second full projection: saves ~57k PE cycles.
- RMS scale applied to raw q/k before rope (rope is norm-preserving and
  commutes with per-head scalars); stats batched into one ln+exp per pair.
- Attention: full-512-wide score matmuls, exp batched in [128,1024] pairs
  (amortizes ACT's per-instruction bubble), causal masking via a Pool-side
  tri multiply on the diagonal squares, y-matmuls restricted to [o:512].
- Softmax sums ride the v ones-column (partition 64/63 of yacc) as in the
  baseline; division is per i-block so the out-projection overlaps the tail
  of attention.
- Engine balance: PE does matmuls only; ACT does exps+stats; DVE does
  PSUM->SBUF copies and PSUM-operand muls; Pool (gpsimd) does SBUF-only
  muls/adds (rope combine, tri); DMA does the rotary shift and v transpose.
"""
import sys
sys.path.insert(0, '/opt/trn_rl_repo')

import numpy as np
import ml_dtypes
from contextlib import ExitStack

import concourse.bass as bass
import concourse.tile as tile
from concourse import bacc, mybir
from concourse.bass_utils import run_bass_kernel_spmd

F32 = mybir.dt.float32
BF = mybir.dt.bfloat16
AF = mybir.ActivationFunctionType

N_HEAD = 16
D_MODEL = 1024
D_HEAD = 64
B, T = 2, 2048
N_CORES = 8
HL = 4              # heads per core
KT = D_MODEL // 128  # 8 contraction tiles
NCH = T // 512      # 4 token chunks
NIB = T // 512      # 4 i-blocks
NTT = T // 128      # 16 j-tiles
SCALE = D_HEAD ** -0.5

_cached = {}


def _build(debug_dump=False):
    nc = bacc.Bacc("TRN2", target_bir_lowering=False, debug=False,
                   num_devices=N_CORES)

    # ---- DRAM I/O ----------------------------------------------------------
    xT = nc.dram_tensor("xT", [D_MODEL, T], BF, kind="ExternalInput").ap()
    wA = nc.dram_tensor("wA", [2, D_MODEL, 384], BF, kind="ExternalInput").ap()
    cosT = nc.dram_tensor("cosT", [128, T], BF, kind="ExternalInput").ap()
    sinNegT = nc.dram_tensor("sinNegT", [128, T], BF,
                             kind="ExternalInput").ap()
    trimask = nc.dram_tensor("trimask", [128, 128], BF,
                             kind="ExternalInput").ap()
    selqk = nc.dram_tensor("selqk", [128, 8, 16], BF,
                           kind="ExternalInput").ap()
    selbc = nc.dram_tensor("selbc", [16, 8, 128], BF,
                           kind="ExternalInput").ap()
    sel4 = nc.dram_tensor("sel4", [4, 2, 128], BF, kind="ExternalInput").ap()
    zpadQ = nc.dram_tensor("zpadQ", [64, T], BF, kind="ExternalInput").ap()
    onescol = nc.dram_tensor("onescol", [128, HL * NTT], BF,
                             kind="ExternalInput").ap()
    wpP = nc.dram_tensor("wpP", [2, 128, 1024], BF, kind="ExternalInput").ap()
    out = nc.dram_tensor("out", [T, D_MODEL], F32, kind="ExternalOutput").ap()
    if debug_dump:
        dbg = {
            "d_qsb": nc.dram_tensor("d_qsb", [2, 128, T], BF,
                                    kind="ExternalOutput").ap(),
            "d_qTz": nc.dram_tensor("d_qTz", [HL, 128, T], BF,
                                    kind="ExternalOutput").ap(),
            "d_kT": nc.dram_tensor("d_kT", [2, 128, T], BF,
                                   kind="ExternalOutput").ap(),
            "d_v": nc.dram_tensor("d_v", [128, 5200], BF,
                                  kind="ExternalOutput").ap(),
            "d_sums": nc.dram_tensor("d_sums", [4, NIB, 512], F32,
                                     kind="ExternalOutput").ap(),
            "d_yP": nc.dram_tensor("d_yP", [2, 128, T], BF,
                                   kind="ExternalOutput").ap(),
        }

    with tile.TileContext(nc) as tc, ExitStack() as ctx:
        ctx.enter_context(nc.allow_low_precision(
            reason="bf16 matmuls/intermediates; tolerance is 2e-2"))

        cpool = ctx.enter_context(tc.tile_pool(name="consts", bufs=1))
        work = ctx.enter_context(tc.tile_pool(name="work", bufs=2))
        ps_s = ctx.enter_context(tc.tile_pool(name="pss", bufs=2,
                                              space="PSUM"))

        # ---- persistent SBUF -----------------------------------------------
        x_sb = cpool.tile([128, KT, T], BF)
        wA_sb = [cpool.tile([128, KT, 384], BF, name=f"wA{p}") for p in (0, 1)]
        cos_sb = cpool.tile([128, T], BF)
        sinNeg_sb = cpool.tile([128, T], BF)
        tri_sb = cpool.tile([128, 128], BF)
        selqk_sb = cpool.tile([128, 8, 16], BF)
        selbc_sb = cpool.tile([16, 8, 128], BF)
        sel4_sb = cpool.tile([4, 2, 128], BF)
        wpP_sb = [cpool.tile([128, 1024], BF, name=f"wpP{p}") for p in (0, 1)]
        qTz = [cpool.tile([128, T], BF, name=f"qTz{h}") for h in range(HL)]
        kT_sb = [cpool.tile([128, T], BF, name=f"kT{p}") for p in (0, 1)]
        # padded past HL*NTT*65+64 so the [p, 2, 1040] transpose-dst view of
        # the last head pair stays in bounds
        v_sb = cpool.tile([128, 5200], BF)
        v3 = v_sb[:, 0:HL * NTT * 65].rearrange("p (g o) -> p g o", o=65)
        yP = [cpool.tile([128, T], BF, name=f"yP{p}") for p in (0, 1)]
        qsbF = [[cpool.tile([128, T], BF, name=f"qsb{p}{m}") for m in (0, 1)]
                for p in (0, 1)]
        qshF = [[cpool.tile([128, T], BF, name=f"qsh{p}{m}") for m in (0, 1)]
                for p in (0, 1)]
        rinv_sb = [cpool.tile([16, 512], BF, name=f"rinv{p}") for p in (0, 1)]
        sums_sb = cpool.tile([4, NIB, 512], F32)
        rinvy_sb = cpool.tile([4, NIB, 512], BF)

        # ---- preamble DMAs -------------------------------------------------
        xr = xT.rearrange("(k p) t -> p k t", p=128)
        nc.sync.dma_start(wA_sb[0][:],
                          wA[0].rearrange("(k p) c -> p k c", p=128))
        for ch in range(NCH):
            nc.sync.dma_start(x_sb[:, :, ch * 512:(ch + 1) * 512],
                              xr[:, :, ch * 512:(ch + 1) * 512])
        nc.sync.dma_start(selqk_sb[:], selqk[:])
        nc.sync.dma_start(wA_sb[1][:],
                          wA[1].rearrange("(k p) c -> p k c", p=128))
        nc.sync.dma_start(cos_sb[:], cosT[:])
        nc.sync.dma_start(sinNeg_sb[:], sinNegT[:])
        nc.sync.dma_start(tri_sb[:], trimask[:])
        nc.sync.dma_start(selbc_sb[:], selbc[:])
        nc.sync.dma_start(sel4_sb[:], sel4[:])
        for h in range(HL):
            half = slice(64, 128) if h % 2 == 0 else slice(0, 64)
            nc.sync.dma_start(qTz[h][half, :], zpadQ[:])
        nc.sync.dma_start(v3[:, :, 64:65], onescol.unsqueeze(2))
        for p in (0, 1):
            nc.sync.dma_start(wpP_sb[p][:], wpP[p])

        ySG_store = {}

        # ---- unit emitters -------------------------------------------------
        def proj_unit(ps_w, hp, m, ch, st_tile):
            cs = slice(ch * 512, (ch + 1) * 512)
            acc = ps_w.tile([128, 512], F32, tag="pa", bufs=2, name="acc")
            for k in range(KT):
                nc.tensor.matmul(acc[:],
                                 wA_sb[hp][:, k, m * 128:(m + 1) * 128],
                                 x_sb[:, k, cs], start=(k == 0),
                                 stop=(k == KT - 1))
            if m < 2:
                dst = qsbF[hp][m][:, cs]
                nc.scalar.copy(dst, acc[:])
                if ch == NCH - 1:
                    # all 4 chunks landed: batched square + 4 stats matmuls
                    sqF = qshF[hp][m]
                    nc.vector.tensor_mul(sqF[:], qsbF[hp][m][:],
                                         qsbF[hp][m][:])
                    for c2 in range(NCH):
                        idx = m * 4 + c2
                        nc.tensor.matmul(
                            st_tile[:], selqk_sb[:, idx, :],
                            sqF[:, c2 * 512:(c2 + 1) * 512],
                            start=(idx == 0), stop=(idx == 7))
            else:
                vdst = work.tile([128, 512], BF, tag="vsb", bufs=3,
                                 name="vdst")
                nc.vector.tensor_copy(vdst[:], acc[:])
                for s4 in range(4):
                    jt = ch * 4 + s4
                    gA = (2 * hp) * NTT + jt
                    vstg = work.tile([128, 128], BF, tag="vstg", bufs=3,
                                     name="vstg")
                    nc.sync.dma_start_transpose(
                        vstg[:], vdst[:, s4 * 128:(s4 + 1) * 128])
                    vv = v_sb[:, gA * 65:gA * 65 + 2080] \
                        .rearrange("p (h x) -> p h x", h=2)[:, :, 0:64]
                    nc.gpsimd.tensor_copy(
                        vv, vstg[:].rearrange("p (h x) -> p h x", h=2))

        def lnexp(hp, st_tile):
            lnt = work.tile([16, 512], F32, tag="lnt", bufs=1, name="lnt")
            nc.scalar.activation(lnt[:], st_tile[:], AF.Ln, scale=1.0 / 64.0)
            nc.scalar.activation(rinv_sb[hp][:], lnt[:], AF.Exp, scale=-0.5)

        def fin_chunks(hp, bcpool, bctag, bcbufs):
            """Chunk closures: rms-scale in place, rotary shift + combine.

            Full-T granularity: t1 overwrites qsbF, t2 overwrites qshF
            (both dead afterwards), so no scratch tiles are needed."""
            chunks = []
            for m in (0, 1):
                for ch in range(NCH):
                    def bc_scale(m=m, ch=ch):
                        cs = slice(ch * 512, (ch + 1) * 512)
                        bc = bcpool.tile([128, 512], F32, tag=bctag,
                                         bufs=bcbufs, name="bc")
                        nc.tensor.matmul(bc[:], selbc_sb[:, m * 4 + ch, :],
                                         rinv_sb[hp][:], start=True,
                                         stop=True)
                        nc.vector.tensor_mul(qsbF[hp][m][:, cs],
                                             qsbF[hp][m][:, cs], bc[:])
                    chunks.append(bc_scale)

                def shifts(m=m):
                    for blk in range(4):
                        d0 = blk * 32
                        s0 = (blk ^ 1) * 32
                        nc.sync.dma_start(qshF[hp][m][d0:d0 + 32, :],
                                          qsbF[hp][m][s0:s0 + 32, :])
                chunks.append(shifts)

            def rope_mul(m, which):
                if which == 0:
                    nc.vector.tensor_mul(qsbF[hp][m][:], qsbF[hp][m][:],
                                         cos_sb[:])
                else:
                    nc.vector.tensor_mul(qshF[hp][m][:], qshF[hp][m][:],
                                         sinNeg_sb[:])

            def rope_add(m):
                if m == 0:
                    nc.vector.tensor_add(qTz[2 * hp][0:64, :],
                                         qsbF[hp][0][0:64, :],
                                         qshF[hp][0][0:64, :])
                    nc.vector.tensor_add(qTz[2 * hp + 1][64:128, :],
                                         qsbF[hp][0][64:128, :],
                                         qshF[hp][0][64:128, :])
                else:
                    nc.vector.tensor_add(kT_sb[hp][:], qsbF[hp][1][:],
                                         qshF[hp][1][:])

            chunks.append(lambda: rope_mul(0, 1))
            chunks.append(lambda: rope_mul(1, 1))
            chunks.append(lambda: rope_mul(0, 0))
            chunks.append(lambda: rope_add(0))
            chunks.append(lambda: rope_mul(1, 0))
            chunks.append(lambda: rope_add(1))
            return chunks

        def attn_gen(ps_o, hp, hh, ib):
            """Generator form: yields after each score/exp batch so two heads
            can interleave on the PE (hides the exp latency)."""
            h_l = 2 * hp + hh
            njt = 4 * (ib + 1)
            ibs = ib * 512
            yacc = ps_o.tile([128, 512], F32, tag="y", bufs=2, name="yacc")

            def ymms(pr, pt):
                for half in (0, 1):
                    jt = 2 * pr + half
                    o = max(0, jt * 128 - ibs)
                    g = h_l * NTT + jt
                    if hh == 0:
                        vau = v_sb[:, g * 65:g * 65 + 128]
                    else:
                        vau = v_sb[:, g * 65 - 64:g * 65 + 64]
                    nc.tensor.matmul(yacc[:, o:512], vau,
                                     pt[:, half * 512 + o:half * 512 + 512],
                                     start=(jt == 0), stop=(jt == njt - 1))

            prev = None
            for pr in range(njt // 2):
                sp = ps_s.tile([128, 1024], F32, tag="s", name="sp")
                for half in (0, 1):
                    jt = 2 * pr + half
                    o_s = jt * 128 - ibs
                    o_s = o_s if 0 < o_s <= 256 else 0
                    nc.tensor.matmul(
                        sp[:, half * 512 + o_s:(half + 1) * 512],
                        kT_sb[hp][:, jt * 128:(jt + 1) * 128],
                        qTz[h_l][:, ibs + o_s:ibs + 512],
                        start=True, stop=True)
                pt = work.tile([128, 1024], BF, tag="p", bufs=4, name="pt")
                nc.scalar.activation(pt[:], sp[:], AF.Exp, scale=SCALE)
                for half in (0, 1):
                    jt = 2 * pr + half
                    o = jt * 128 - ibs
                    if o >= 0:
                        lo = half * 512 + o
                        nc.vector.tensor_mul(pt[:, lo:lo + 128],
                                             pt[:, lo:lo + 128], tri_sb[:])
                if prev is not None:
                    ymms(*prev)
                prev = (pr, pt)
                yield
            ymms(*prev)

            ySG = work.tile([128, 512], F32, tag="ysg", bufs=12, name="ySG")
            nc.vector.tensor_copy(ySG[:], yacc[:])
            srow = 64 if hh == 0 else 63
            nc.sync.dma_start(sums_sb[2 * hp + hh:2 * hp + hh + 1, ib, :],
                              ySG[srow:srow + 1, :])
            ySG_store[(hp, hh, ib)] = ySG

        def attn_pair(ps_o, hp, ib):
            g0 = attn_gen(ps_o, hp, 0, ib)
            g1 = attn_gen(ps_o, hp, 1, ib)
            for _ in range(4 * (ib + 1) // 2 + 1):
                next(g0, None)
                next(g1, None)

        def recip_unit(ib):
            # one reciprocal per i-block covers both pairs; split into 4
            # chunks so attention's DVE ops can slot between them
            for q in range(4):
                nc.vector.reciprocal(rinvy_sb[:, ib, q * 128:(q + 1) * 128],
                                     sums_sb[:, ib, q * 128:(q + 1) * 128])

        def ydiv_unit(ps_o, hp, ib):
            ibs = ib * 512
            bc2 = ps_o.tile([128, 512], F32, tag="oa", bufs=2, name="bc2")
            nc.tensor.matmul(bc2[:], sel4_sb[:, hp, :], rinvy_sb[:, ib, :],
                             start=True, stop=True)
            e = ySG_store[(hp, 0, ib)]
            o_ = ySG_store[(hp, 1, ib)]
            nc.vector.tensor_mul(yP[hp][0:64, ibs:ibs + 512], e[0:64, :],
                                 bc2[0:64, :])
            nc.vector.tensor_mul(yP[hp][64:128, ibs:ibs + 512],
                                 o_[64:128, :], bc2[64:128, :])

        def o_unit(ps_o, ib):
            for mt in range(4 * ib, 4 * ib + 4):
                ost = work.tile([128, 1024], F32, tag="ost", name="ost")
                for oc in (0, 1):
                    oa = ps_o.tile([128, 512], F32, tag="oa", bufs=2,
                                   name="oa")
                    for t in (0, 1):
                        nc.tensor.matmul(oa[:],
                                         yP[t][:, mt * 128:(mt + 1) * 128],
                                         wpP_sb[t][:, oc * 512:(oc + 1) * 512],
                                         start=(t == 0), stop=(t == 1))
                    dst = ost[:, oc * 512:(oc + 1) * 512]
                    if ib == NIB - 1 and oc == 1:
                        nc.scalar.copy(dst, oa[:])
                    else:
                        nc.vector.tensor_copy(dst, oa[:])
                nc.sync.dma_start(out[mt * 128:(mt + 1) * 128, :], ost[:])

        # ---- emission ------------------------------------------------------
        with tc.tile_pool(name="psw", bufs=1, space="PSUM") as ps_w:
            st0 = ps_w.tile([16, 512], F32, tag="st", name="st0")
            for m in range(3):
                for ch in range(NCH):
                    proj_unit(ps_w, 0, m, ch, st0)
            lnexp(0, st0)

            # fin(p0) interleaved with P(p1)
            st1 = ps_w.tile([16, 512], F32, tag="st", name="st1")
            p1_units = [(m, ch) for m in range(3) for ch in range(NCH)]
            f0 = fin_chunks(0, ps_w, "bcw", 1)
            fi = 0
            for i, (m, ch) in enumerate(p1_units):
                proj_unit(ps_w, 1, m, ch, st1)
                take = ((i + 1) * len(f0)) // len(p1_units) - fi
                for _ in range(take):
                    f0[fi]()
                    fi += 1
            lnexp(1, st1)

        # attention; psw's banks are free, so yacc is double-buffered and
        # the oa/bc2 ring fits alongside the score ring
        with tc.tile_pool(name="pso", bufs=2, space="PSUM") as ps_o:
            # A(pair0) interleaved with fin(p1)
            f1 = fin_chunks(1, ps_o, "oa", 2)
            fi = 0
            for i in range(NIB):
                take = ((i + 1) * len(f1)) // NIB - fi
                for _ in range(take):
                    f1[fi]()
                    fi += 1
                attn_pair(ps_o, 0, i)

            # pair1 attention + division + out-projection, lagged one i-block
            for ib in range(NIB):
                attn_pair(ps_o, 1, ib)
                recip_unit(ib)
                if ib >= 1:
                    ydiv_unit(ps_o, 0, ib - 1)
                    ydiv_unit(ps_o, 1, ib - 1)
                    o_unit(ps_o, ib - 1)
            ydiv_unit(ps_o, 0, NIB - 1)
            ydiv_unit(ps_o, 1, NIB - 1)
            o_unit(ps_o, NIB - 1)

        if debug_dump:
            for p in (0, 1):
                nc.sync.dma_start(dbg["d_qsb"][p], qsbF[p][0][:])
                nc.sync.dma_start(dbg["d_kT"][p], kT_sb[p][:])
                nc.sync.dma_start(dbg["d_yP"][p], yP[p][:])
            nc.sync.dma_start(dbg["d_sums"], sums_sb[:])
            for h in range(HL):
                nc.sync.dma_start(dbg["d_qTz"][h], qTz[h][:])
            nc.sync.dma_start(dbg["d_v"], v_sb[:])

    nc.compile()
    return nc


def _host_inputs(x, w_attn, w_proj):
    """Build the 8 per-core input maps (bf16 device tensors)."""
    bf = ml_dtypes.bfloat16
    inv_freq = 1.0 / (10000.0 ** (np.arange(0, D_HEAD, 2, dtype=np.float32)
                                  / D_HEAD))
    t = np.arange(T, dtype=np.float32)
    freqs = np.einsum('i,j->ij', t, inv_freq)            # [T, 32]
    emb = np.concatenate([freqs, freqs], 1)              # [T, 64]
    cos64 = np.cos(emb).T                                # [64, T]
    sin64 = np.sin(emb).T
    sgn = np.where(np.arange(64) < 32, -1.0, 1.0)[:, None].astype(np.float32)
    sinNeg64 = sin64 * sgn
    cosT = np.concatenate([cos64, cos64], 0).astype(bf)  # [128, T]
    sinNegT = np.concatenate([sinNeg64, sinNeg64], 0).astype(bf)

    tri = (np.arange(128)[:, None] <= np.arange(128)[None, :]).astype(bf)

    selqk = np.zeros((128, 8, 16), np.float32)
    selbc = np.zeros((16, 8, 128), np.float32)
    for m in range(2):
        for ch in range(NCH):
            u = m * 4 + ch
            for p in range(128):
                c = 4 * ch + 2 * m + (1 if p >= 64 else 0)
                selqk[p, u, c] = 1.0
                selbc[c, u, p] = 1.0
    sel4 = np.zeros((4, 2, 128), np.float32)
    for p in (0, 1):
        sel4[2 * p, p, 0:64] = 1.0
        sel4[2 * p + 1, p, 64:128] = 1.0

    zpadQ = np.zeros((64, T), np.float32)
    onescol = np.ones((128, HL * NTT), np.float32)

    wq = w_attn[:D_MODEL]
    wk = w_attn[D_MODEL:2 * D_MODEL]
    wv_full = w_attn[2 * D_MODEL:]

    in_maps = []
    for c in range(N_CORES):
        b, hg = c // 4, c % 4
        wA = np.zeros((2, D_MODEL, 384), np.float32)
        for hp in (0, 1):
            hs = slice((hg * 4 + 2 * hp) * D_HEAD,
                       (hg * 4 + 2 * hp + 2) * D_HEAD)     # 128 rows
            wA[hp, :, 0:128] = wq[hs].T
            wA[hp, :, 128:256] = wk[hs].T
            wA[hp, :, 256:384] = wv_full[hs].T
        wp_c = [w_proj[:, (hg * 4 + j) * D_HEAD:(hg * 4 + j + 1) * D_HEAD].T
                for j in range(HL)]
        wpP = np.stack([np.concatenate([wp_c[0], wp_c[1]], 0),
                        np.concatenate([wp_c[2], wp_c[3]], 0)])

        in_maps.append({
            "xT": np.ascontiguousarray(x[b].T).astype(bf),
            "wA": wA.astype(bf),
            "cosT": cosT, "sinNegT": sinNegT, "trimask": tri,
            "selqk": selqk.astype(bf), "selbc": selbc.astype(bf),
            "sel4": sel4.astype(bf),
            "zpadQ": zpadQ.astype(bf), "onescol": onescol.astype(bf),
            "wpP": wpP.astype(bf),
        })
    return in_maps


def kernel(x, w_attn, w_proj, _want_results=False):
    x = np.asarray(x, dtype=np.float32)
    w_attn = np.asarray(w_attn, dtype=np.float32)
    w_proj = np.asarray(w_proj, dtype=np.float32)

    if "nc" not in _cached:
        _cached["nc"] = _build()
    nc = _cached["nc"]

    in_maps = _host_inputs(x, w_attn, w_proj)
    res = run_bass_kernel_spmd(nc, in_maps, list(range(N_CORES)))

    full = np.zeros((B, T, D_MODEL), np.float32)
    for c in range(N_CORES):
        full[c // 4] += res.results[c]["out"]
    if _want_results:
        return full, res
    return full
